# revision 23
# baseline (speedup 1.0000x reference)
"""Trainium2 Bass kernel for a 2-layer Mistral-style VLM block (TP-8 + sequence-parallel).

v2 strategy (vs v1 AllReduce baseline):
- LoRA rank-8 + rmsnorm weights folded into base weights on host (f32).
- Tensor parallel: core r holds Q heads 4r..4r+3, KV head r, DFF slice r,
  Wo/Wd k-sharded. Weights bf16 in contiguous per-output-tile slabs
  (one big DMA per slab). Moving operands bf16/f32r -> 1 cyc/row.
- Sequence-parallel residual: core r owns tokens 96r..96(r+1) of each batch;
  h stays in SBUF f32 the whole run. Per block: local rmsnorm stats ->
  AllGather x (bf16, flat [8][4096][96]) -> GEMMs over the full 768-token
  batch -> partial outputs (bf16, flat [8][4096][96]) -> ReduceScatter back
  to own tokens. 1/rms row AllGathered separately, applied post-GEMM.
- Projector token-sharded per core (full VH contraction, zero-padded cols),
  no collective; proj bias pre-added into hinit on host.
- Per-batch processing double-streams compute vs collectives.
"""

import sys

sys.path.insert(0, '/opt/trn_rl_repo')

import numpy as np
import ml_dtypes

NCORES = 8
D, VH, DFF, NL, VOCAB, NH, NKV, HD, RK, SCALE = 4096, 1024, 14336, 2, 32000, 32, 8, 128, 8, 4.0
B, NIMG, T = 2, 257, 511
S = NIMG + T            # 768 tokens per batch
NTOK = B * S
TG = S // NCORES        # 96 own tokens per batch
KT = D // 128           # 32
FT = DFF // NCORES // 128   # 14
QH = NH // NCORES       # 4
VKT = VH // 128         # 8
CH = 384
NCH = S // CH           # 2
JT = S // 128           # 6
HK = KT // 2        # 16 k-tiles per collective half
EPS = 1e-5
ISQ = 1.0 / float(np.sqrt(HD))
EXP_BIAS = -10.0
MASK_NEG = -1e30

BF16 = ml_dtypes.bfloat16
F16NP = np.float16
_PROGRAM = None


def _bf(x):
    return np.ascontiguousarray(np.asarray(x, np.float32).astype(BF16))


def _h(x):
    return np.ascontiguousarray(np.asarray(x, np.float32).astype(F16NP))


def _r(x):
    """fp32 -> fp32r RNE rounding (11 explicit mantissa bits)."""
    u = np.ascontiguousarray(x, np.float32).view(np.uint32)
    low = u & np.uint32(0xFFF)
    hi = u >> np.uint32(12)
    carry = (low > 0x800) | ((low == 0x800) & ((hi & 1) == 1))
    return ((hi + carry.astype(np.uint32)) << np.uint32(12)).view(np.float32)


def _slab(W):
    """[128m, D2] weight block -> stationary slab [128 contraction, k*128+m]."""
    n = W.shape[1]
    return np.ascontiguousarray(
        W.T.reshape(n // 128, 128, 128).transpose(1, 0, 2).reshape(128, n))


def _build_program():
    import concourse.bass as bass
    import concourse.bacc as bacc
    import concourse.mybir as mybir
    import concourse.tile as tile

    F32 = mybir.dt.float32
    F32R = mybir.dt.float32r
    BF = mybir.dt.bfloat16
    F16 = mybir.dt.float16
    AF = mybir.ActivationFunctionType
    ALU = mybir.AluOpType

    nc = bacc.Bacc("TRN2", target_bir_lowering=False)

    hinit_in = nc.dram_tensor("hinit", [128, KT * 2 * TG], F32, kind="ExternalInput")
    imgx_in = nc.dram_tensor("imgx", [128, VKT * 2 * TG], F16, kind="ExternalInput")
    projw_in = nc.dram_tensor("projw", [KT, 128, VH], F16, kind="ExternalInput")
    cos_in = nc.dram_tensor("cos_t", [128, S], F32, kind="ExternalInput")
    sin_in = nc.dram_tensor("sin_t", [128, S], F32, kind="ExternalInput")   # sign-folded
    mask_in = nc.dram_tensor("mask6", [6, 128, CH], BF, kind="ExternalInput")
    onesb_in = nc.dram_tensor("onesb", [128, 1], BF, kind="ExternalInput")
    onesh_in = nc.dram_tensor("onesh", [128, 1], F16, kind="ExternalInput")
    identb_in = nc.dram_tensor("identb", [128, 128], BF, kind="ExternalInput")
    lnf_in = nc.dram_tensor("lnf", [128, KT], F32, kind="ExternalInput")
    wqkv_in = [nc.dram_tensor(f"wqkv{l}", [6, 128, D], F16, kind="ExternalInput") for l in range(NL)]
    wo_in = [nc.dram_tensor(f"wo{l}", [KT, 128, QH * 128], F16, kind="ExternalInput") for l in range(NL)]
    wgu_in = [nc.dram_tensor(f"wgu{l}", [2 * FT, 128, D], F16, kind="ExternalInput") for l in range(NL)]
    wd_in = [nc.dram_tensor(f"wd{l}", [KT, 128, FT * 128], F16, kind="ExternalInput") for l in range(NL)]
    out_ext = nc.dram_tensor("out", [128, KT * 2 * TG], F32, kind="ExternalOutput")

    RG = [list(range(NCORES))]

    with tile.TileContext(nc) as tc:
        with tc.tile_pool(name="sb", bufs=1) as sb, \
             tc.tile_pool(name="ps", bufs=1, space="PSUM") as ps, \
             tc.tile_pool(name="dram", bufs=1, space="DRAM") as dram:

            # ---- resident constants ----
            cos_sb = sb.tile([128, S], F32, tag="res_cos", bufs=1)
            sin_sb = sb.tile([128, S], F32, tag="res_sin", bufs=1)
            onesb_sb = sb.tile([128, 1], BF, tag="res_onesb", bufs=1)
            onesh_sb = sb.tile([128, 1], F16, tag="res_onesh", bufs=1)
            identb_sb = sb.tile([128, 128], BF, tag="res_identb", bufs=1)
            lnf_sb = sb.tile([128, KT], F32, tag="res_lnf", bufs=1)
            for t_, i_ in [(cos_sb, cos_in), (sin_sb, sin_in), (onesb_sb, onesb_in), (onesh_sb, onesh_in),
                           (identb_sb, identb_in), (lnf_sb, lnf_in)]:
                nc.sync.dma_start(t_[:], i_[:])
            mask_sb = []
            for j in range(6):
                mt_ = sb.tile([128, CH], BF, tag=f"res_mask{j}", bufs=1, name=f"msk{j}")
                nc.sync.dma_start(mt_[:], mask_in[j])
                mask_sb.append(mt_)
            eps_sb = sb.tile([128, 1], F32, tag="res_eps", bufs=1)
            nb_sb = sb.tile([128, 1], F32, tag="res_nb", bufs=1)
            nc.vector.memset(eps_sb[:], EPS)
            nc.vector.memset(nb_sb[:], EXP_BIAS)

            # ---- persistent activations ----
            hloc = sb.tile([128, KT * 2 * TG], F32, tag="hloc", bufs=1, name="hloc")
            nc.sync.dma_start(hloc[:], hinit_in[:])
            xmega = sb.tile([128, KT * S], F16, tag="xmega", bufs=1, name="xmega")
            mtm = sb.tile([128, FT * S], F16, tag="mtm", bufs=1, name="mtm")

            def hc(b, k):
                return slice(k * 2 * TG + b * TG, k * 2 * TG + (b + 1) * TG)

            # ---- phase 0: token-sharded projector, no collective ----
            # imgx: own tokens' image embeds, [VH rows as 8 k-tiles][2*TG cols],
            # zero cols where own token is not an img token. out += proj_W @ imgx.
            imgx_sb = sb.tile([128, VKT * 2 * TG], F16, tag="imgsb", bufs=1, name="imgx_sb")
            nc.sync.dma_start(imgx_sb[:], imgx_in[:])
            for m in range(KT):
                pw = sb.tile([128, VH], F16, tag="wslab", bufs=3, padded_shape=[128, D],
                             name=f"pw{m}")
                nc.scalar.dma_start(pw[:], projw_in[m])
                pt = ps.tile([128, 2 * TG], F32, tag="ps1", bufs=4, name=f"pj{m}")
                for k in range(VKT):
                    nc.tensor.matmul(pt[:], pw[:, k * 128:(k + 1) * 128],
                                     imgx_sb[:, k * 2 * TG:(k + 1) * 2 * TG],
                                     start=(k == 0), stop=(k == VKT - 1))
                pa = sb.tile([128, 2 * TG], F32, tag="hn", bufs=3, name=f"pa{m}")
                nc.vector.tensor_tensor(pa[:], hloc[:, m * 2 * TG:(m + 1) * 2 * TG],
                                        pt[:], ALU.add)
                nc.scalar.activation(hloc[:, m * 2 * TG:(m + 1) * 2 * TG], pa[:], AF.Copy)

            # ---- helpers ----
            def bcast_row(row_ap, width, nm):
                rd = dram.tile([1, width], F32, tag="rowd", bufs=4, name=f"rd{nm}")
                nc.sync.dma_start(rd[:], row_ap)
                bc = sb.tile([128, width], F32, tag="bc" + str(width), bufs=2, name=f"bct{nm}")
                nc.sync.dma_start(bc[:], rd[:].to_broadcast((128, width)))
                return bc

            def norm_ag(l, site, b, delta, last=False):
                """h update (+delta) for own tokens of batch b, ssq, AllGather x
                (bf16) + 1/rms row. Returns (agout, scout) dram tiles, or the
                output write if last=True."""
                nm = f"{l}{site}{b}"
                agin = [dram.tile([D // 2, TG], F16, tag="agin", bufs=4,
                                  name=f"agi{nm}{h}") for h in range(2)]
                agout = [dram.tile([NCORES, D // 2, TG], F16, tag="agout", bufs=4,
                                   name=f"ago{nm}{h}", addr_space="Shared")
                         for h in range(2)]
                ssq = ps.tile([1, TG], F32, tag="psS", bufs=2, name=f"ssq{nm}")
                if not last:
                    xball = sb.tile([128, KT * TG], F16, tag="xball", bufs=2,
                                    name=f"xba{nm}")
                for k in range(KT):
                    cols = hc(b, k)
                    if delta is not None:
                        dh, dk = (delta[0], k) if k < HK else (delta[1], k - HK)
                        dr = sb.tile([128, TG], F16, tag="drt", bufs=3, name=f"dr{nm}{k}")
                        nc.sync.dma_start(dr[:], dh[dk * 128:(dk + 1) * 128, :])
                        hn = sb.tile([128, TG], F32, tag="hn", bufs=3, name=f"hn{nm}{k}")
                        nc.vector.tensor_tensor(hn[:], hloc[:, cols], dr[:], ALU.add)
                        nc.scalar.activation(hloc[:, cols], hn[:], AF.Copy)
                        hsrc = hn[:]
                    else:
                        hsrc = hloc[:, cols]
                    sq = sb.tile([128, TG], F16, tag="sq", bufs=3, name=f"sq{nm}{k}")
                    nc.scalar.activation(sq[:], hsrc, AF.Square)
                    nc.tensor.matmul(ssq[:], onesh_sb[:], sq[:],
                                     start=(k == 0), stop=(k == KT - 1))
                    if not last:
                        nc.vector.tensor_scalar(xball[:, k * TG:(k + 1) * TG], hsrc,
                                                1.0, None, ALU.mult)
                        if k == HK - 1 or k == KT - 1:
                            h = 0 if k < HK else 1
                            nc.sync.dma_start(
                                agin[h][:].rearrange("(k p) t -> p k t", p=128),
                                xball[:, h * HK * TG:(h + 1) * HK * TG])
                            nc.gpsimd.collective_compute(
                                "AllGather", ALU.bypass, replica_groups=RG,
                                ins=[agin[h].opt()], outs=[agout[h].opt()])
                s_sb = sb.tile([1, TG], F32, tag="scal", bufs=4, name=f"ss{nm}")
                r_sb = sb.tile([1, TG], F32, tag="scal", bufs=4, name=f"sr{nm}")
                nc.scalar.activation(s_sb[:], ssq[:], AF.Sqrt, scale=1.0 / D,
                                     bias=eps_sb[0:1, :])
                nc.vector.reciprocal(r_sb[:], s_sb[:])
                if last:
                    # final norm: scale updated hloc, write own tokens out
                    bcl = bcast_row(r_sb[:], TG, f"f{nm}")
                    for k in range(KT):
                        cols = hc(b, k)
                        ot = sb.tile([128, TG], F32, tag="hn", bufs=3, name=f"ot{nm}{k}")
                        nc.vector.scalar_tensor_tensor(ot[:], hloc[:, cols],
                                                       lnf_sb[:, k:k + 1],
                                                       bcl[:], ALU.mult, ALU.mult)
                        nc.sync.dma_start(out_ext[:, cols], ot[:])
                    return None, None
                scin = dram.tile([1, TG], F32, tag="scin", bufs=2, name=f"sci{nm}")
                scout = dram.tile([1, S], F32, tag="scout", bufs=2, name=f"sco{nm}")
                nc.sync.dma_start(scin[:], r_sb[:])
                nc.gpsimd.collective_compute("AllGather", ALU.bypass, replica_groups=RG,
                                             ins=[scin.opt()], outs=[scout.opt()])
                return agout, scout

            def fill_x(agout, nm):
                """agout [8][D][TG] bf16 -> xmega [128, k*S + 96g+i]."""
                for k in range(KT):
                    ah, ak = (agout[0], k) if k < HK else (agout[1], k - HK)
                    nc.scalar.dma_start(
                        xmega[:, k * S:(k + 1) * S],
                        ah[:, ak * 128:(ak + 1) * 128, :].rearrange("g p t -> p g t"))

            def rs_site(l, site, b, emit_tiles):
                """emit_tiles: iterator of (m, c, psum_tile, scale_or_None).
                Evacuates partials (bf16) into [8][D][TG] and ReduceScatters."""
                nm = f"{l}{site}{b}"
                rsin = [dram.tile([NCORES, D // 2, TG], F16, tag="rsin", bufs=4,
                                  name=f"ri{nm}{h}") for h in range(2)]
                rsout = [dram.tile([D // 2, TG], F16, tag="rsout", bufs=4,
                                   name=f"ro{nm}{h}") for h in range(2)]
                ng = CH // TG
                for (m, c, pt, bcs) in emit_tiles:
                    ev = sb.tile([128, CH], F16, tag="evb", bufs=3, name=f"ev{nm}{m}{c}")
                    if bcs is None:
                        nc.vector.tensor_scalar(ev[:], pt[:], 1.0, None, ALU.mult)
                    else:
                        nc.vector.tensor_tensor(ev[:], pt[:], bcs, ALU.mult)
                    h, mh = (0, m) if m < HK else (1, m - HK)
                    dst = rsin[h][c * ng:(c + 1) * ng, mh * 128:(mh + 1) * 128, :]
                    nc.sync.dma_start(dst.rearrange("g p t -> p g t"), ev[:])
                    if m == HK - 1 and c == NCH - 1:
                        nc.gpsimd.collective_compute(
                            "ReduceScatter", ALU.add, replica_groups=RG,
                            ins=[rsin[0].opt()], outs=[rsout[0].opt()])
                nc.gpsimd.collective_compute("ReduceScatter", ALU.add, replica_groups=RG,
                                             ins=[rsin[1].opt()], outs=[rsout[1].opt()])
                return rsout

            def attn_block(l, b, agout, scout):
                """QKV + rope + attention + Wo partial -> ReduceScatter tile."""
                nm = f"a{l}{b}"
                fill_x(agout, nm)
                bc = bcast_row(scout[:], S, nm)
                qkv = []
                for m in range(6):
                    sl = sb.tile([128, D], F16, tag="wslab", bufs=3, name=f"qs{nm}{m}")
                    nc.scalar.dma_start(sl[:], wqkv_in[l][m])
                    if m < 5:
                        qraw = sb.tile([128, S], F32, tag="t32", bufs=4, name=f"qr{nm}{m}")
                    out_bf = sb.tile([128, S], F16 if m < 5 else BF, tag="qkv", bufs=6, name=f"qo{nm}{m}")
                    pts = [ps.tile([128, CH], F32, tag="ps1", bufs=4, name=f"qp{nm}{m}{c}")
                           for c in range(NCH)]
                    for k in range(KT):
                        for c in range(NCH):
                            nc.tensor.matmul(
                                pts[c][:], sl[:, k * 128:(k + 1) * 128],
                                xmega[:, k * S + c * CH: k * S + (c + 1) * CH],
                                start=(k == 0), stop=(k == KT - 1))
                    for c in range(NCH):
                        if m < 5:
                            nc.scalar.activation(qraw[:, c * CH:(c + 1) * CH], pts[c][:], AF.Copy)
                        else:
                            nc.vector.tensor_tensor(out_bf[:, c * CH:(c + 1) * CH], pts[c][:],
                                                    bc[:, c * CH:(c + 1) * CH], ALU.mult)
                    if m < 5:
                        qs = sb.tile([128, S], F32, tag="t32", bufs=4, name=f"qh{nm}{m}")
                        nc.sync.dma_start(qs[0:64, :], qraw[64:128, :])
                        nc.sync.dma_start(qs[64:128, :], qraw[0:64, :])
                        t2 = sb.tile([128, S], F32, tag="t32", bufs=4, name=f"t2{nm}{m}")
                        nc.vector.tensor_tensor(t2[:], qraw[:], cos_sb[:], ALU.mult)
                        u2 = sb.tile([128, S], F32, tag="t32", bufs=4, name=f"u2{nm}{m}")
                        nc.vector.tensor_tensor(u2[:], qs[:], sin_sb[:], ALU.mult)
                        q3 = sb.tile([128, S], F32, tag="t32", bufs=4, name=f"q3{nm}{m}")
                        nc.vector.tensor_tensor(q3[:], t2[:], u2[:], ALU.add)
                        nc.vector.tensor_tensor(out_bf[:], q3[:], bc[:], ALU.mult)
                    qkv.append(out_bf)

                vtok = []
                for t in range(JT):
                    trp = ps.tile([128, 128], BF, tag="ps1", bufs=4, name=f"vt{nm}{t}")
                    nc.tensor.transpose(trp[:], qkv[5][:, t * 128:(t + 1) * 128],
                                        identb_sb[:])
                    vt = sb.tile([128, 128], BF, tag="vtok", bufs=6, name=f"vk{nm}{t}")
                    nc.scalar.activation(vt[:], trp[:], AF.Copy)
                    vtok.append(vt)

                amega = sb.tile([128, QH * S], F16, tag="amega", bufs=1, name=f"am{nm}")
                ksb = qkv[4]
                for hh in range(QH):
                    qh_t = qkv[hh]
                    for c in range(NCH):
                        njt = 3 * (c + 1)
                        ap_ps = ps.tile([128, CH], F32, tag="psA", bufs=2, name=f"ap{nm}{hh}{c}")
                        ss_ps = ps.tile([1, CH], F32, tag="psS", bufs=2, name=f"sm{nm}{hh}{c}")
                        for jt in range(njt):
                            sc = ps.tile([128, CH], F32, tag="ps1", bufs=4,
                                         name=f"sc{nm}{hh}{c}{jt}")
                            nc.tensor.matmul(sc[:], ksb[:, jt * 128:(jt + 1) * 128],
                                             qh_t[:, c * CH:(c + 1) * CH],
                                             start=True, stop=True)
                            et = sb.tile([128, CH], BF, tag="expT", bufs=4,
                                         name=f"et{nm}{hh}{c}{jt}")
                            if jt >= 3 * c:
                                madd = sb.tile([128, CH], F32, tag="madd", bufs=3,
                                               name=f"md{nm}{hh}{c}{jt}")
                                nc.vector.tensor_tensor(madd[:], sc[:], mask_sb[jt][:],
                                                        ALU.add)
                                nc.scalar.activation(et[:], madd[:], AF.Exp, scale=ISQ,
                                                     bias=nb_sb[:])
                            else:
                                nc.scalar.activation(et[:], sc[:], AF.Exp, scale=ISQ,
                                                     bias=nb_sb[:])
                            nc.tensor.matmul(ss_ps[:], onesb_sb[:], et[:],
                                             start=(jt == 0), stop=(jt == njt - 1))
                            nc.tensor.matmul(ap_ps[:], vtok[jt][:], et[:],
                                             start=(jt == 0), stop=(jt == njt - 1))
                        rec = sb.tile([1, CH], F32, tag="scal", bufs=4, name=f"rc{nm}{hh}{c}")
                        nc.vector.reciprocal(rec[:], ss_ps[:])
                        rbc = bcast_row(rec[:], CH, f"r{nm}{hh}{c}")
                        nc.vector.tensor_tensor(
                            amega[:, hh * S + c * CH: hh * S + (c + 1) * CH],
                            ap_ps[:], rbc[:], ALU.mult)

                def wo_tiles():
                    for m in range(KT):
                        sl = sb.tile([128, QH * 128], F16, tag="wslab", bufs=3,
                                     padded_shape=[128, D], name=f"wos{nm}{m}")
                        nc.scalar.dma_start(sl[:], wo_in[l][m])
                        pts = [ps.tile([128, CH], F32, tag="ps1", bufs=4,
                                       name=f"op{nm}{m}{c}") for c in range(NCH)]
                        for k in range(QH):
                            for c in range(NCH):
                                nc.tensor.matmul(
                                    pts[c][:], sl[:, k * 128:(k + 1) * 128],
                                    amega[:, k * S + c * CH: k * S + (c + 1) * CH],
                                    start=(k == 0), stop=(k == QH - 1))
                        for c in range(NCH):
                            yield (m, c, pts[c], None)
                return rs_site(l, 'a', b, wo_tiles())

            def mlp_block(l, b, agout, scout):
                nm = f"m{l}{b}"
                fill_x(agout, nm)
                bc = bcast_row(scout[:], S, nm)
                for j in range(FT):
                    gsb = sb.tile([128, S], F32, tag="t32", bufs=4, name=f"gs{nm}{j}")
                    usb = sb.tile([128, S], F32, tag="t32", bufs=4, name=f"us{nm}{j}")
                    for gu in range(2):
                        sl = sb.tile([128, D], F16, tag="wslab", bufs=3, name=f"gsl{nm}{j}{gu}")
                        nc.scalar.dma_start(sl[:], wgu_in[l][2 * j + gu])
                        dst = gsb if gu == 0 else usb
                        pts = [ps.tile([128, CH], F32, tag="ps1", bufs=4,
                                       name=f"g{nm}{j}{gu}{c}") for c in range(NCH)]
                        for k in range(KT):
                            for c in range(NCH):
                                nc.tensor.matmul(
                                    pts[c][:], sl[:, k * 128:(k + 1) * 128],
                                    xmega[:, k * S + c * CH: k * S + (c + 1) * CH],
                                    start=(k == 0), stop=(k == KT - 1))
                        for c in range(NCH):
                            nc.vector.tensor_tensor(dst[:, c * CH:(c + 1) * CH], pts[c][:],
                                                    bc[:, c * CH:(c + 1) * CH], ALU.mult)
                    sil = sb.tile([128, S], F32, tag="t32", bufs=4, name=f"si{nm}{j}")
                    nc.scalar.activation(sil[:], gsb[:], AF.Silu)
                    nc.vector.tensor_tensor(mtm[:, j * S:(j + 1) * S], sil[:], usb[:],
                                            ALU.mult)

                def wd_tiles():
                    for m in range(KT):
                        sl = sb.tile([128, FT * 128], F16, tag="wslab", bufs=3,
                                     padded_shape=[128, D], name=f"wds{nm}{m}")
                        nc.scalar.dma_start(sl[:], wd_in[l][m])
                        pts = [ps.tile([128, CH], F32, tag="ps1", bufs=4,
                                       name=f"dp{nm}{m}{c}") for c in range(NCH)]
                        for k in range(FT):
                            for c in range(NCH):
                                nc.tensor.matmul(
                                    pts[c][:], sl[:, k * 128:(k + 1) * 128],
                                    mtm[:, k * S + c * CH: k * S + (c + 1) * CH],
                                    start=(k == 0), stop=(k == FT - 1))
                        for c in range(NCH):
                            yield (m, c, pts[c], None)
                return rs_site(l, 'm', b, wd_tiles())

            # ---- main schedule ----
            rs_prev = [None, None]          # pending MLP RS from previous layer
            for l in range(NL):
                ags = [None, None]
                for b in range(B):
                    ags[b] = norm_ag(l, 'a', b, rs_prev[b])
                rs_a = [None, None]
                for b in range(B):
                    rs_a[b] = attn_block(l, b, ags[b][0], ags[b][1])
                for b in range(B):
                    ags[b] = norm_ag(l, 'm', b, rs_a[b])
                for b in range(B):
                    rs_prev[b] = mlp_block(l, b, ags[b][0], ags[b][1])
                    if l == NL - 1:
                        norm_ag(NL, 'f', b, rs_prev[b], last=True)

    nc.compile()
    return nc


def _host_prep(inputs):
    I = {k: np.asarray(v) for k, v in inputs.items()}

    def fold(W, A, Bm, lnw=None):
        Wf = W.astype(np.float32) + np.float32(SCALE) * (
            Bm.astype(np.float32) @ A.astype(np.float32))
        if lnw is not None:
            Wf = Wf * lnw.astype(np.float32)[None, :]
        return Wf

    ids = np.asarray(I['input_ids'], np.int64)
    embed = I['embed'].astype(np.float32)
    proj_b = I['proj_b'].astype(np.float32)

    # initial h per batch: [B, S, D]; img positions get proj bias (GEMM adds the rest)
    h0 = np.empty((B, S, D), np.float32)
    h0[:, :NIMG, :] = proj_b[None, None, :]
    h0[:, NIMG:, :] = embed[ids]

    inv = 1.0 / (10000.0 ** (np.arange(0, HD, 2, dtype=np.float64) / HD))
    ang = np.arange(S, dtype=np.float64)[:, None] * inv[None, :]
    cosT = np.ascontiguousarray(np.concatenate([np.cos(ang), np.cos(ang)], 1).T).astype(np.float32)
    sinT = np.ascontiguousarray(np.concatenate([-np.sin(ang), np.sin(ang)], 1).T).astype(np.float32)

    mask6 = np.zeros((6, 128, CH), np.float32)
    for jt in range(6):
        c = 0 if jt < 3 else 1
        jj = np.arange(jt * 128, (jt + 1) * 128)[:, None]
        ii = np.arange(c * CH, (c + 1) * CH)[None, :]
        mask6[jt] = np.where(jj <= ii, 0.0, MASK_NEG)

    imgT = I['image_embeds'].astype(np.float32)     # [B, NIMG, VH]
    projW = I['proj_W'].astype(np.float32)          # [D, VH]
    # proj slabs: [m][kk, vh_k*128+mm] = projW[m*128+mm, vh_k*128+kk]
    projw_slab = np.stack([_slab(projW[m * 128:(m + 1) * 128, :]) for m in range(KT)])

    shared = dict(
        cos_t=cosT, sin_t=sinT, mask6=_bf(mask6),
        onesb=_bf(np.ones((128, 1), np.float32)),
        onesh=_h(np.ones((128, 1), np.float32)),
        identb=_bf(np.eye(128, dtype=np.float32)),
        lnf=np.ascontiguousarray(I['ln_f'].astype(np.float32).reshape(KT, 128).T),
        projw=_h(projw_slab),
    )

    per_core = [dict(shared) for _ in range(NCORES)]
    for r in range(NCORES):
        t0 = r * TG
        # hinit [128, k*2TG + b*TG + i] = h0[b, t0+i, k*128+p]
        own = h0[:, t0:t0 + TG, :]                       # [B, TG, D]
        per_core[r]["hinit"] = np.ascontiguousarray(
            own.reshape(B, TG, KT, 128).transpose(3, 2, 0, 1).reshape(128, KT * 2 * TG))
        # imgx: [128, vk*2TG + b*TG + i] = img_embed[b, t0+i, vk*128+p] or 0
        ix = np.zeros((B, TG, VH), np.float32)
        w = max(0, min(TG, NIMG - t0))
        if w > 0:
            ix[:, :w, :] = imgT[:, t0:t0 + w, :]
        per_core[r]["imgx"] = _h(np.ascontiguousarray(
            ix.reshape(B, TG, VKT, 128).transpose(3, 2, 0, 1).reshape(128, VKT * 2 * TG)))

    for l in range(NL):
        Wq = fold(I['Wq'][l], I['Aq'][l], I['Bq'][l], I['ln1'][l])
        Wk = fold(I['Wk'][l], I['Ak'][l], I['Bk'][l], I['ln1'][l])
        Wv = fold(I['Wv'][l], I['Av'][l], I['Bv'][l], I['ln1'][l])
        Wo = fold(I['Wo'][l], I['Ao'][l], I['Bo'][l])
        Wg = fold(I['Wg'][l], I['Ag'][l], I['Bg'][l], I['ln2'][l])
        Wu = fold(I['Wu'][l], I['Au'][l], I['Bu'][l], I['ln2'][l])
        Wd = fold(I['Wd'][l], I['Ad'][l], I['Bd'][l])
        for r in range(NCORES):
            wl = np.vstack([Wq[r * 512:(r + 1) * 512],
                            Wk[r * HD:(r + 1) * HD],
                            Wv[r * HD:(r + 1) * HD]])          # [768, D]
            per_core[r][f"wqkv{l}"] = _h(np.stack(
                [_slab(wl[m * 128:(m + 1) * 128, :]) for m in range(6)]))
            wo_l = Wo[:, r * 512:(r + 1) * 512]                # [D, 512]
            per_core[r][f"wo{l}"] = _h(np.stack(
                [_slab(wo_l[m * 128:(m + 1) * 128, :]) for m in range(KT)]))
            gu = np.empty((2 * FT * 128, D), np.float32)
            gsh = Wg[r * FT * 128:(r + 1) * FT * 128]
            ush = Wu[r * FT * 128:(r + 1) * FT * 128]
            for j in range(FT):
                gu[(2 * j) * 128:(2 * j + 1) * 128] = gsh[j * 128:(j + 1) * 128]
                gu[(2 * j + 1) * 128:(2 * j + 2) * 128] = ush[j * 128:(j + 1) * 128]
            per_core[r][f"wgu{l}"] = _h(np.stack(
                [_slab(gu[m * 128:(m + 1) * 128, :]) for m in range(2 * FT)]))
            wd_l = Wd[:, r * FT * 128:(r + 1) * FT * 128]      # [D, 1792]
            per_core[r][f"wd{l}"] = _h(np.stack(
                [_slab(wd_l[m * 128:(m + 1) * 128, :]) for m in range(KT)]))
    return per_core


def kernel(**inputs):
    global _PROGRAM
    from concourse.bass_utils import run_bass_kernel_spmd

    in_maps = _host_prep(inputs)
    if _PROGRAM is None:
        _PROGRAM = _build_program()
    res = None
    for attempt in range(3):
        try:
            res = run_bass_kernel_spmd(_PROGRAM, in_maps, list(range(NCORES)))
            break
        except Exception as e:
            if attempt == 2 or 'UNAVAILABLE' not in str(type(e).__name__) + str(e):
                raise
    out = np.empty((B, S, D), np.float32)
    for r in range(NCORES):
        o = np.asarray(res.results[r]["out"], np.float32)      # [128, KT*2*TG]
        o = o.reshape(128, KT, B, TG).transpose(2, 3, 1, 0).reshape(B, TG, D)
        out[:, r * TG:(r + 1) * TG, :] = o
    return out


# revision 24
# speedup vs baseline: 1.0112x; 1.0112x over previous
"""Trainium2 Bass kernel for a 2-layer Mistral-style VLM block (TP-8 + sequence-parallel).

v2 strategy (vs v1 AllReduce baseline):
- LoRA rank-8 + rmsnorm weights folded into base weights on host (f32).
- Tensor parallel: core r holds Q heads 4r..4r+3, KV head r, DFF slice r,
  Wo/Wd k-sharded. Weights bf16 in contiguous per-output-tile slabs
  (one big DMA per slab). Moving operands bf16/f32r -> 1 cyc/row.
- Sequence-parallel residual: core r owns tokens 96r..96(r+1) of each batch;
  h stays in SBUF f32 the whole run. Per block: local rmsnorm stats ->
  AllGather x (bf16, flat [8][4096][96]) -> GEMMs over the full 768-token
  batch -> partial outputs (bf16, flat [8][4096][96]) -> ReduceScatter back
  to own tokens. 1/rms row AllGathered separately, applied post-GEMM.
- Projector token-sharded per core (full VH contraction, zero-padded cols),
  no collective; proj bias pre-added into hinit on host.
- Per-batch processing double-streams compute vs collectives.
"""

import sys

sys.path.insert(0, '/opt/trn_rl_repo')

import numpy as np
import ml_dtypes

NCORES = 8
D, VH, DFF, NL, VOCAB, NH, NKV, HD, RK, SCALE = 4096, 1024, 14336, 2, 32000, 32, 8, 128, 8, 4.0
B, NIMG, T = 2, 257, 511
S = NIMG + T            # 768 tokens per batch
NTOK = B * S
TG = S // NCORES        # 96 own tokens per batch
KT = D // 128           # 32
FT = DFF // NCORES // 128   # 14
QH = NH // NCORES       # 4
VKT = VH // 128         # 8
CH = 384
NCH = S // CH           # 2
JT = S // 128           # 6
HK = KT // 2        # 16 k-tiles per collective half
EPS = 1e-5
ISQ = 1.0 / float(np.sqrt(HD))
EXP_BIAS = -10.0
MASK_NEG = -1e30

BF16 = ml_dtypes.bfloat16
F16NP = np.float16
_PROGRAM = None


def _bf(x):
    return np.ascontiguousarray(np.asarray(x, np.float32).astype(BF16))


def _h(x):
    return np.ascontiguousarray(np.asarray(x, np.float32).astype(F16NP))


def _r(x):
    """fp32 -> fp32r RNE rounding (11 explicit mantissa bits)."""
    u = np.ascontiguousarray(x, np.float32).view(np.uint32)
    low = u & np.uint32(0xFFF)
    hi = u >> np.uint32(12)
    carry = (low > 0x800) | ((low == 0x800) & ((hi & 1) == 1))
    return ((hi + carry.astype(np.uint32)) << np.uint32(12)).view(np.float32)


def _slab(W):
    """[128m, D2] weight block -> stationary slab [128 contraction, k*128+m]."""
    n = W.shape[1]
    return np.ascontiguousarray(
        W.T.reshape(n // 128, 128, 128).transpose(1, 0, 2).reshape(128, n))


def _build_program():
    import concourse.bass as bass
    import concourse.bacc as bacc
    import concourse.mybir as mybir
    import concourse.tile as tile

    F32 = mybir.dt.float32
    F32R = mybir.dt.float32r
    BF = mybir.dt.bfloat16
    F16 = mybir.dt.float16
    AF = mybir.ActivationFunctionType
    ALU = mybir.AluOpType

    nc = bacc.Bacc("TRN2", target_bir_lowering=False)

    hinit_in = nc.dram_tensor("hinit", [128, KT * 2 * TG], F32, kind="ExternalInput")
    imgx_in = nc.dram_tensor("imgx", [128, VKT * 2 * TG], F16, kind="ExternalInput")
    projw_in = nc.dram_tensor("projw", [KT, 128, VH], F16, kind="ExternalInput")
    cos_in = nc.dram_tensor("cos_t", [128, S], F32, kind="ExternalInput")
    sin_in = nc.dram_tensor("sin_t", [128, S], F32, kind="ExternalInput")   # sign-folded
    mask_in = nc.dram_tensor("mask6", [6, 128, CH], BF, kind="ExternalInput")
    onesb_in = nc.dram_tensor("onesb", [128, 1], BF, kind="ExternalInput")
    onesh_in = nc.dram_tensor("onesh", [128, 1], F16, kind="ExternalInput")
    identb_in = nc.dram_tensor("identb", [128, 128], BF, kind="ExternalInput")
    lnf_in = nc.dram_tensor("lnf", [128, KT], F32, kind="ExternalInput")
    wqkv_in = [nc.dram_tensor(f"wqkv{l}", [6, 128, D], F16, kind="ExternalInput") for l in range(NL)]
    wo_in = [nc.dram_tensor(f"wo{l}", [KT, 128, QH * 128], F16, kind="ExternalInput") for l in range(NL)]
    wgu_in = [nc.dram_tensor(f"wgu{l}", [2 * FT, 128, D], F16, kind="ExternalInput") for l in range(NL)]
    wd_in = [nc.dram_tensor(f"wd{l}", [KT, 128, FT * 128], F16, kind="ExternalInput") for l in range(NL)]
    out_ext = nc.dram_tensor("out", [128, KT * 2 * TG], F32, kind="ExternalOutput")

    RG = [list(range(NCORES))]

    with tile.TileContext(nc) as tc:
        with tc.tile_pool(name="sb", bufs=1) as sb, \
             tc.tile_pool(name="ps", bufs=1, space="PSUM") as ps, \
             tc.tile_pool(name="dram", bufs=1, space="DRAM") as dram:

            # ---- resident constants ----
            cos_sb = sb.tile([128, S], F32, tag="res_cos", bufs=1)
            sin_sb = sb.tile([128, S], F32, tag="res_sin", bufs=1)
            onesb_sb = sb.tile([128, 1], BF, tag="res_onesb", bufs=1)
            onesh_sb = sb.tile([128, 1], F16, tag="res_onesh", bufs=1)
            identb_sb = sb.tile([128, 128], BF, tag="res_identb", bufs=1)
            lnf_sb = sb.tile([128, KT], F32, tag="res_lnf", bufs=1)
            for t_, i_ in [(cos_sb, cos_in), (sin_sb, sin_in), (onesb_sb, onesb_in), (onesh_sb, onesh_in),
                           (identb_sb, identb_in), (lnf_sb, lnf_in)]:
                nc.sync.dma_start(t_[:], i_[:])
            mask_sb = []
            for j in range(6):
                mt_ = sb.tile([128, CH], BF, tag=f"res_mask{j}", bufs=1, name=f"msk{j}")
                nc.sync.dma_start(mt_[:], mask_in[j])
                mask_sb.append(mt_)
            eps_sb = sb.tile([128, 1], F32, tag="res_eps", bufs=1)
            nb_sb = sb.tile([128, 1], F32, tag="res_nb", bufs=1)
            nc.vector.memset(eps_sb[:], EPS)
            nc.vector.memset(nb_sb[:], EXP_BIAS)

            # ---- persistent activations ----
            hloc = sb.tile([128, KT * 2 * TG], F32, tag="hloc", bufs=1, name="hloc")
            nc.sync.dma_start(hloc[:], hinit_in[:])
            xmega = sb.tile([128, KT * S], F16, tag="xmega", bufs=1, name="xmega")
            mtm = sb.tile([128, FT * S], F16, tag="mtm", bufs=1, name="mtm")

            def hc(b, k):
                return slice(k * 2 * TG + b * TG, k * 2 * TG + (b + 1) * TG)

            # ---- phase 0: token-sharded projector, no collective ----
            # imgx: own tokens' image embeds, [VH rows as 8 k-tiles][2*TG cols],
            # zero cols where own token is not an img token. out += proj_W @ imgx.
            imgx_sb = sb.tile([128, VKT * 2 * TG], F16, tag="imgsb", bufs=1, name="imgx_sb")
            nc.sync.dma_start(imgx_sb[:], imgx_in[:])
            for m in range(KT):
                pw = sb.tile([128, VH], F16, tag="wslab", bufs=3, padded_shape=[128, D],
                             name=f"pw{m}")
                nc.scalar.dma_start(pw[:], projw_in[m])
                pt = ps.tile([128, 2 * TG], F32, tag="ps1", bufs=4, name=f"pj{m}")
                for k in range(VKT):
                    nc.tensor.matmul(pt[:], pw[:, k * 128:(k + 1) * 128],
                                     imgx_sb[:, k * 2 * TG:(k + 1) * 2 * TG],
                                     start=(k == 0), stop=(k == VKT - 1))
                pa = sb.tile([128, 2 * TG], F32, tag="hn", bufs=3, name=f"pa{m}")
                nc.vector.tensor_tensor(pa[:], hloc[:, m * 2 * TG:(m + 1) * 2 * TG],
                                        pt[:], ALU.add)
                nc.scalar.activation(hloc[:, m * 2 * TG:(m + 1) * 2 * TG], pa[:], AF.Copy)

            # ---- helpers ----
            def bcast_row(row_ap, width, nm):
                rd = dram.tile([1, width], F32, tag="rowd", bufs=4, name=f"rd{nm}")
                nc.sync.dma_start(rd[:], row_ap)
                bc = sb.tile([128, width], F32, tag="bc" + str(width), bufs=2, name=f"bct{nm}")
                nc.sync.dma_start(bc[:], rd[:].to_broadcast((128, width)))
                return bc

            def norm_ag(l, site, b, delta, last=False):
                """h update (+delta) for own tokens of batch b, ssq, AllGather x
                (bf16) + 1/rms row. Returns (agout, scout) dram tiles, or the
                output write if last=True."""
                nm = f"{l}{site}{b}"
                agin = [dram.tile([D // 2, TG], F16, tag="agin", bufs=4,
                                  name=f"agi{nm}{h}") for h in range(2)]
                agout = [dram.tile([NCORES, D // 2, TG], F16, tag="agout", bufs=4,
                                   name=f"ago{nm}{h}", addr_space="Shared")
                         for h in range(2)]
                ssq = ps.tile([1, TG], F32, tag="psS", bufs=2, name=f"ssq{nm}")
                if not last:
                    xball = sb.tile([128, KT * TG], F16, tag="xball", bufs=2,
                                    name=f"xba{nm}")
                for k in range(KT):
                    cols = hc(b, k)
                    if delta is not None:
                        dh, dk = (delta[0], k) if k < HK else (delta[1], k - HK)
                        dr = sb.tile([128, TG], F16, tag="drt", bufs=3, name=f"dr{nm}{k}")
                        nc.sync.dma_start(dr[:], dh[dk * 128:(dk + 1) * 128, :])
                        hn = sb.tile([128, TG], F32, tag="hn", bufs=3, name=f"hn{nm}{k}")
                        nc.vector.tensor_tensor(hn[:], hloc[:, cols], dr[:], ALU.add)
                        nc.scalar.activation(hloc[:, cols], hn[:], AF.Copy)
                        hsrc = hn[:]
                    else:
                        hsrc = hloc[:, cols]
                    sq = sb.tile([128, TG], F16, tag="sq", bufs=3, name=f"sq{nm}{k}")
                    nc.scalar.activation(sq[:], hsrc, AF.Square)
                    nc.tensor.matmul(ssq[:], onesh_sb[:], sq[:],
                                     start=(k == 0), stop=(k == KT - 1))
                    if not last:
                        nc.vector.tensor_scalar(xball[:, k * TG:(k + 1) * TG], hsrc,
                                                1.0, None, ALU.mult)
                        if k == HK - 1 or k == KT - 1:
                            h = 0 if k < HK else 1
                            nc.sync.dma_start(
                                agin[h][:].rearrange("(k p) t -> p k t", p=128),
                                xball[:, h * HK * TG:(h + 1) * HK * TG])
                            nc.gpsimd.collective_compute(
                                "AllGather", ALU.bypass, replica_groups=RG,
                                ins=[agin[h].opt()], outs=[agout[h].opt()])
                s_sb = sb.tile([1, TG], F32, tag="scal", bufs=4, name=f"ss{nm}")
                r_sb = sb.tile([1, TG], F32, tag="scal", bufs=4, name=f"sr{nm}")
                nc.scalar.activation(s_sb[:], ssq[:], AF.Sqrt, scale=1.0 / D,
                                     bias=eps_sb[0:1, :])
                nc.vector.reciprocal(r_sb[:], s_sb[:])
                if last:
                    # final norm: scale updated hloc, write own tokens out
                    bcl = bcast_row(r_sb[:], TG, f"f{nm}")
                    for k in range(KT):
                        cols = hc(b, k)
                        ot = sb.tile([128, TG], F32, tag="hn", bufs=3, name=f"ot{nm}{k}")
                        nc.vector.scalar_tensor_tensor(ot[:], hloc[:, cols],
                                                       lnf_sb[:, k:k + 1],
                                                       bcl[:], ALU.mult, ALU.mult)
                        nc.sync.dma_start(out_ext[:, cols], ot[:])
                    return None, None
                scin = dram.tile([1, TG], F32, tag="scin", bufs=2, name=f"sci{nm}")
                scout = dram.tile([1, S], F32, tag="scout", bufs=2, name=f"sco{nm}")
                nc.sync.dma_start(scin[:], r_sb[:])
                nc.gpsimd.collective_compute("AllGather", ALU.bypass, replica_groups=RG,
                                             ins=[scin.opt()], outs=[scout.opt()])
                return agout, scout

            def fill_x(agout, nm):
                """agout [8][D][TG] bf16 -> xmega [128, k*S + 96g+i]."""
                for k in range(KT):
                    ah, ak = (agout[0], k) if k < HK else (agout[1], k - HK)
                    nc.scalar.dma_start(
                        xmega[:, k * S:(k + 1) * S],
                        ah[:, ak * 128:(ak + 1) * 128, :].rearrange("g p t -> p g t"))

            def rs_site(l, site, b, emit_tiles):
                """emit_tiles: iterator of (m, c, psum_tile, scale_or_None).
                Evacuates partials (bf16) into [8][D][TG] and ReduceScatters."""
                nm = f"{l}{site}{b}"
                rsin = [dram.tile([NCORES, D // 2, TG], F16, tag="rsin", bufs=4,
                                  name=f"ri{nm}{h}") for h in range(2)]
                rsout = [dram.tile([D // 2, TG], F16, tag="rsout", bufs=4,
                                   name=f"ro{nm}{h}") for h in range(2)]
                ng = CH // TG
                for (m, c, pt, bcs) in emit_tiles:
                    ev = sb.tile([128, CH], F16, tag="evb", bufs=3, name=f"ev{nm}{m}{c}")
                    if bcs is None:
                        nc.vector.tensor_scalar(ev[:], pt[:], 1.0, None, ALU.mult)
                    else:
                        nc.vector.tensor_tensor(ev[:], pt[:], bcs, ALU.mult)
                    h, mh = (0, m) if m < HK else (1, m - HK)
                    dst = rsin[h][c * ng:(c + 1) * ng, mh * 128:(mh + 1) * 128, :]
                    nc.sync.dma_start(dst.rearrange("g p t -> p g t"), ev[:])
                    if m == HK - 1 and c == NCH - 1:
                        nc.gpsimd.collective_compute(
                            "ReduceScatter", ALU.add, replica_groups=RG,
                            ins=[rsin[0].opt()], outs=[rsout[0].opt()])
                nc.gpsimd.collective_compute("ReduceScatter", ALU.add, replica_groups=RG,
                                             ins=[rsin[1].opt()], outs=[rsout[1].opt()])
                return rsout

            def attn_block(l, b, agout, scout):
                """QKV + rope + attention + Wo partial -> ReduceScatter tile."""
                nm = f"a{l}{b}"
                fill_x(agout, nm)
                bc = bcast_row(scout[:], S, nm)
                qkv = []
                for m in range(6):
                    sl = sb.tile([128, D], F16, tag="wslab", bufs=3, name=f"qs{nm}{m}")
                    nc.scalar.dma_start(sl[:], wqkv_in[l][m])
                    if m < 5:
                        qraw = sb.tile([128, S], F32, tag="t32", bufs=4, name=f"qr{nm}{m}")
                    out_bf = sb.tile([128, S], F16 if m < 5 else BF, tag="qkv", bufs=6, name=f"qo{nm}{m}")
                    for c in range(NCH):
                        pt = ps.tile([128, CH], F32, tag="ps1", bufs=4, name=f"qp{nm}{m}{c}")
                        for k in range(KT):
                            nc.tensor.matmul(
                                pt[:], sl[:, k * 128:(k + 1) * 128],
                                xmega[:, k * S + c * CH: k * S + (c + 1) * CH],
                                start=(k == 0), stop=(k == KT - 1))
                        if m < 5:
                            nc.scalar.activation(qraw[:, c * CH:(c + 1) * CH], pt[:], AF.Copy)
                        else:
                            nc.vector.tensor_tensor(out_bf[:, c * CH:(c + 1) * CH], pt[:],
                                                    bc[:, c * CH:(c + 1) * CH], ALU.mult)
                    if m < 5:
                        qs = sb.tile([128, S], F32, tag="t32", bufs=4, name=f"qh{nm}{m}")
                        nc.sync.dma_start(qs[0:64, :], qraw[64:128, :])
                        nc.sync.dma_start(qs[64:128, :], qraw[0:64, :])
                        t2 = sb.tile([128, S], F32, tag="t32", bufs=4, name=f"t2{nm}{m}")
                        nc.vector.tensor_tensor(t2[:], qraw[:], cos_sb[:], ALU.mult)
                        u2 = sb.tile([128, S], F32, tag="t32", bufs=4, name=f"u2{nm}{m}")
                        nc.vector.tensor_tensor(u2[:], qs[:], sin_sb[:], ALU.mult)
                        q3 = sb.tile([128, S], F32, tag="t32", bufs=4, name=f"q3{nm}{m}")
                        nc.vector.tensor_tensor(q3[:], t2[:], u2[:], ALU.add)
                        nc.vector.tensor_tensor(out_bf[:], q3[:], bc[:], ALU.mult)
                    qkv.append(out_bf)

                vtok = []
                for t in range(JT):
                    trp = ps.tile([128, 128], BF, tag="ps1", bufs=4, name=f"vt{nm}{t}")
                    nc.tensor.transpose(trp[:], qkv[5][:, t * 128:(t + 1) * 128],
                                        identb_sb[:])
                    vt = sb.tile([128, 128], BF, tag="vtok", bufs=6, name=f"vk{nm}{t}")
                    nc.scalar.activation(vt[:], trp[:], AF.Copy)
                    vtok.append(vt)

                amega = sb.tile([128, QH * S], F16, tag="amega", bufs=1, name=f"am{nm}")
                ksb = qkv[4]
                for hh in range(QH):
                    qh_t = qkv[hh]
                    for c in range(NCH):
                        njt = 3 * (c + 1)
                        ap_ps = ps.tile([128, CH], F32, tag="psA", bufs=2, name=f"ap{nm}{hh}{c}")
                        ss_ps = ps.tile([1, CH], F32, tag="psS", bufs=2, name=f"sm{nm}{hh}{c}")
                        for jt in range(njt):
                            sc = ps.tile([128, CH], F32, tag="ps1", bufs=4,
                                         name=f"sc{nm}{hh}{c}{jt}")
                            nc.tensor.matmul(sc[:], ksb[:, jt * 128:(jt + 1) * 128],
                                             qh_t[:, c * CH:(c + 1) * CH],
                                             start=True, stop=True)
                            et = sb.tile([128, CH], BF, tag="expT", bufs=4,
                                         name=f"et{nm}{hh}{c}{jt}")
                            if jt >= 3 * c:
                                madd = sb.tile([128, CH], F32, tag="madd", bufs=3,
                                               name=f"md{nm}{hh}{c}{jt}")
                                nc.vector.tensor_tensor(madd[:], sc[:], mask_sb[jt][:],
                                                        ALU.add)
                                nc.scalar.activation(et[:], madd[:], AF.Exp, scale=ISQ,
                                                     bias=nb_sb[:])
                            else:
                                nc.scalar.activation(et[:], sc[:], AF.Exp, scale=ISQ,
                                                     bias=nb_sb[:])
                            nc.tensor.matmul(ss_ps[:], onesb_sb[:], et[:],
                                             start=(jt == 0), stop=(jt == njt - 1))
                            nc.tensor.matmul(ap_ps[:], vtok[jt][:], et[:],
                                             start=(jt == 0), stop=(jt == njt - 1))
                        rec = sb.tile([1, CH], F32, tag="scal", bufs=4, name=f"rc{nm}{hh}{c}")
                        nc.vector.reciprocal(rec[:], ss_ps[:])
                        rbc = bcast_row(rec[:], CH, f"r{nm}{hh}{c}")
                        nc.vector.tensor_tensor(
                            amega[:, hh * S + c * CH: hh * S + (c + 1) * CH],
                            ap_ps[:], rbc[:], ALU.mult)

                def wo_tiles():
                    for m in range(KT):
                        sl = sb.tile([128, QH * 128], F16, tag="wslab", bufs=3,
                                     padded_shape=[128, D], name=f"wos{nm}{m}")
                        nc.scalar.dma_start(sl[:], wo_in[l][m])
                        for c in range(NCH):
                            pt = ps.tile([128, CH], F32, tag="ps1", bufs=4,
                                         name=f"op{nm}{m}{c}")
                            for k in range(QH):
                                nc.tensor.matmul(
                                    pt[:], sl[:, k * 128:(k + 1) * 128],
                                    amega[:, k * S + c * CH: k * S + (c + 1) * CH],
                                    start=(k == 0), stop=(k == QH - 1))
                            yield (m, c, pt, None)
                return rs_site(l, 'a', b, wo_tiles())

            def mlp_block(l, b, agout, scout):
                nm = f"m{l}{b}"
                fill_x(agout, nm)
                bc = bcast_row(scout[:], S, nm)
                for j in range(FT):
                    gsb = sb.tile([128, S], F32, tag="t32", bufs=4, name=f"gs{nm}{j}")
                    usb = sb.tile([128, S], F32, tag="t32", bufs=4, name=f"us{nm}{j}")
                    for gu in range(2):
                        sl = sb.tile([128, D], F16, tag="wslab", bufs=3, name=f"gsl{nm}{j}{gu}")
                        nc.scalar.dma_start(sl[:], wgu_in[l][2 * j + gu])
                        dst = gsb if gu == 0 else usb
                        for c in range(NCH):
                            pt = ps.tile([128, CH], F32, tag="ps1", bufs=4,
                                         name=f"g{nm}{j}{gu}{c}")
                            for k in range(KT):
                                nc.tensor.matmul(
                                    pt[:], sl[:, k * 128:(k + 1) * 128],
                                    xmega[:, k * S + c * CH: k * S + (c + 1) * CH],
                                    start=(k == 0), stop=(k == KT - 1))
                            nc.vector.tensor_tensor(dst[:, c * CH:(c + 1) * CH], pt[:],
                                                    bc[:, c * CH:(c + 1) * CH], ALU.mult)
                    sil = sb.tile([128, S], F32, tag="t32", bufs=4, name=f"si{nm}{j}")
                    nc.scalar.activation(sil[:], gsb[:], AF.Silu)
                    nc.vector.tensor_tensor(mtm[:, j * S:(j + 1) * S], sil[:], usb[:],
                                            ALU.mult)

                def wd_tiles():
                    for m in range(KT):
                        sl = sb.tile([128, FT * 128], F16, tag="wslab", bufs=3,
                                     padded_shape=[128, D], name=f"wds{nm}{m}")
                        nc.scalar.dma_start(sl[:], wd_in[l][m])
                        for c in range(NCH):
                            pt = ps.tile([128, CH], F32, tag="ps1", bufs=4,
                                         name=f"dp{nm}{m}{c}")
                            for k in range(FT):
                                nc.tensor.matmul(
                                    pt[:], sl[:, k * 128:(k + 1) * 128],
                                    mtm[:, k * S + c * CH: k * S + (c + 1) * CH],
                                    start=(k == 0), stop=(k == FT - 1))
                            yield (m, c, pt, None)
                return rs_site(l, 'm', b, wd_tiles())

            # ---- main schedule ----
            rs_prev = [None, None]          # pending MLP RS from previous layer
            for l in range(NL):
                ags = [None, None]
                for b in range(B):
                    ags[b] = norm_ag(l, 'a', b, rs_prev[b])
                rs_a = [None, None]
                for b in range(B):
                    rs_a[b] = attn_block(l, b, ags[b][0], ags[b][1])
                for b in range(B):
                    ags[b] = norm_ag(l, 'm', b, rs_a[b])
                for b in range(B):
                    rs_prev[b] = mlp_block(l, b, ags[b][0], ags[b][1])
                    if l == NL - 1:
                        norm_ag(NL, 'f', b, rs_prev[b], last=True)

    nc.compile()
    return nc


def _host_prep(inputs):
    I = {k: np.asarray(v) for k, v in inputs.items()}

    def fold(W, A, Bm, lnw=None):
        Wf = W.astype(np.float32) + np.float32(SCALE) * (
            Bm.astype(np.float32) @ A.astype(np.float32))
        if lnw is not None:
            Wf = Wf * lnw.astype(np.float32)[None, :]
        return Wf

    ids = np.asarray(I['input_ids'], np.int64)
    embed = I['embed'].astype(np.float32)
    proj_b = I['proj_b'].astype(np.float32)

    # initial h per batch: [B, S, D]; img positions get proj bias (GEMM adds the rest)
    h0 = np.empty((B, S, D), np.float32)
    h0[:, :NIMG, :] = proj_b[None, None, :]
    h0[:, NIMG:, :] = embed[ids]

    inv = 1.0 / (10000.0 ** (np.arange(0, HD, 2, dtype=np.float64) / HD))
    ang = np.arange(S, dtype=np.float64)[:, None] * inv[None, :]
    cosT = np.ascontiguousarray(np.concatenate([np.cos(ang), np.cos(ang)], 1).T).astype(np.float32)
    sinT = np.ascontiguousarray(np.concatenate([-np.sin(ang), np.sin(ang)], 1).T).astype(np.float32)

    mask6 = np.zeros((6, 128, CH), np.float32)
    for jt in range(6):
        c = 0 if jt < 3 else 1
        jj = np.arange(jt * 128, (jt + 1) * 128)[:, None]
        ii = np.arange(c * CH, (c + 1) * CH)[None, :]
        mask6[jt] = np.where(jj <= ii, 0.0, MASK_NEG)

    imgT = I['image_embeds'].astype(np.float32)     # [B, NIMG, VH]
    projW = I['proj_W'].astype(np.float32)          # [D, VH]
    # proj slabs: [m][kk, vh_k*128+mm] = projW[m*128+mm, vh_k*128+kk]
    projw_slab = np.stack([_slab(projW[m * 128:(m + 1) * 128, :]) for m in range(KT)])

    shared = dict(
        cos_t=cosT, sin_t=sinT, mask6=_bf(mask6),
        onesb=_bf(np.ones((128, 1), np.float32)),
        onesh=_h(np.ones((128, 1), np.float32)),
        identb=_bf(np.eye(128, dtype=np.float32)),
        lnf=np.ascontiguousarray(I['ln_f'].astype(np.float32).reshape(KT, 128).T),
        projw=_h(projw_slab),
    )

    per_core = [dict(shared) for _ in range(NCORES)]
    for r in range(NCORES):
        t0 = r * TG
        # hinit [128, k*2TG + b*TG + i] = h0[b, t0+i, k*128+p]
        own = h0[:, t0:t0 + TG, :]                       # [B, TG, D]
        per_core[r]["hinit"] = np.ascontiguousarray(
            own.reshape(B, TG, KT, 128).transpose(3, 2, 0, 1).reshape(128, KT * 2 * TG))
        # imgx: [128, vk*2TG + b*TG + i] = img_embed[b, t0+i, vk*128+p] or 0
        ix = np.zeros((B, TG, VH), np.float32)
        w = max(0, min(TG, NIMG - t0))
        if w > 0:
            ix[:, :w, :] = imgT[:, t0:t0 + w, :]
        per_core[r]["imgx"] = _h(np.ascontiguousarray(
            ix.reshape(B, TG, VKT, 128).transpose(3, 2, 0, 1).reshape(128, VKT * 2 * TG)))

    for l in range(NL):
        Wq = fold(I['Wq'][l], I['Aq'][l], I['Bq'][l], I['ln1'][l])
        Wk = fold(I['Wk'][l], I['Ak'][l], I['Bk'][l], I['ln1'][l])
        Wv = fold(I['Wv'][l], I['Av'][l], I['Bv'][l], I['ln1'][l])
        Wo = fold(I['Wo'][l], I['Ao'][l], I['Bo'][l])
        Wg = fold(I['Wg'][l], I['Ag'][l], I['Bg'][l], I['ln2'][l])
        Wu = fold(I['Wu'][l], I['Au'][l], I['Bu'][l], I['ln2'][l])
        Wd = fold(I['Wd'][l], I['Ad'][l], I['Bd'][l])
        for r in range(NCORES):
            wl = np.vstack([Wq[r * 512:(r + 1) * 512],
                            Wk[r * HD:(r + 1) * HD],
                            Wv[r * HD:(r + 1) * HD]])          # [768, D]
            per_core[r][f"wqkv{l}"] = _h(np.stack(
                [_slab(wl[m * 128:(m + 1) * 128, :]) for m in range(6)]))
            wo_l = Wo[:, r * 512:(r + 1) * 512]                # [D, 512]
            per_core[r][f"wo{l}"] = _h(np.stack(
                [_slab(wo_l[m * 128:(m + 1) * 128, :]) for m in range(KT)]))
            gu = np.empty((2 * FT * 128, D), np.float32)
            gsh = Wg[r * FT * 128:(r + 1) * FT * 128]
            ush = Wu[r * FT * 128:(r + 1) * FT * 128]
            for j in range(FT):
                gu[(2 * j) * 128:(2 * j + 1) * 128] = gsh[j * 128:(j + 1) * 128]
                gu[(2 * j + 1) * 128:(2 * j + 2) * 128] = ush[j * 128:(j + 1) * 128]
            per_core[r][f"wgu{l}"] = _h(np.stack(
                [_slab(gu[m * 128:(m + 1) * 128, :]) for m in range(2 * FT)]))
            wd_l = Wd[:, r * FT * 128:(r + 1) * FT * 128]      # [D, 1792]
            per_core[r][f"wd{l}"] = _h(np.stack(
                [_slab(wd_l[m * 128:(m + 1) * 128, :]) for m in range(KT)]))
    return per_core


def kernel(**inputs):
    global _PROGRAM
    from concourse.bass_utils import run_bass_kernel_spmd

    in_maps = _host_prep(inputs)
    if _PROGRAM is None:
        _PROGRAM = _build_program()
    res = None
    for attempt in range(3):
        try:
            res = run_bass_kernel_spmd(_PROGRAM, in_maps, list(range(NCORES)))
            break
        except Exception as e:
            if attempt == 2 or 'UNAVAILABLE' not in str(type(e).__name__) + str(e):
                raise
    out = np.empty((B, S, D), np.float32)
    for r in range(NCORES):
        o = np.asarray(res.results[r]["out"], np.float32)      # [128, KT*2*TG]
        o = o.reshape(128, KT, B, TG).transpose(2, 3, 1, 0).reshape(B, TG, D)
        out[:, r * TG:(r + 1) * TG, :] = o
    return out


# revision 25
# speedup vs baseline: 1.0445x; 1.0329x over previous
"""Trainium2 Bass kernel for a 2-layer Mistral-style VLM block (TP-8 + sequence-parallel).

v2 strategy (vs v1 AllReduce baseline):
- LoRA rank-8 + rmsnorm weights folded into base weights on host (f32).
- Tensor parallel: core r holds Q heads 4r..4r+3, KV head r, DFF slice r,
  Wo/Wd k-sharded. Weights bf16 in contiguous per-output-tile slabs
  (one big DMA per slab). Moving operands bf16/f32r -> 1 cyc/row.
- Sequence-parallel residual: core r owns tokens 96r..96(r+1) of each batch;
  h stays in SBUF f32 the whole run. Per block: local rmsnorm stats ->
  AllGather x (bf16, flat [8][4096][96]) -> GEMMs over the full 768-token
  batch -> partial outputs (bf16, flat [8][4096][96]) -> ReduceScatter back
  to own tokens. 1/rms row AllGathered separately, applied post-GEMM.
- Projector token-sharded per core (full VH contraction, zero-padded cols),
  no collective; proj bias pre-added into hinit on host.
- Per-batch processing double-streams compute vs collectives.
"""

import sys

sys.path.insert(0, '/opt/trn_rl_repo')

import numpy as np
import ml_dtypes

NCORES = 8
D, VH, DFF, NL, VOCAB, NH, NKV, HD, RK, SCALE = 4096, 1024, 14336, 2, 32000, 32, 8, 128, 8, 4.0
B, NIMG, T = 2, 257, 511
S = NIMG + T            # 768 tokens per batch
NTOK = B * S
TG = S // NCORES        # 96 own tokens per batch
KT = D // 128           # 32
FT = DFF // NCORES // 128   # 14
QH = NH // NCORES       # 4
VKT = VH // 128         # 8
CH = 384
NCH = S // CH           # 2
JT = S // 128           # 6
HK = KT // 2        # 16 k-tiles per collective half
EPS = 1e-5
ISQ = 1.0 / float(np.sqrt(HD))
EXP_BIAS = -10.0
MASK_NEG = -1e30

BF16 = ml_dtypes.bfloat16
F16NP = np.float16
_PROGRAM = None


def _bf(x):
    return np.ascontiguousarray(np.asarray(x, np.float32).astype(BF16))


def _h(x):
    return np.ascontiguousarray(np.asarray(x, np.float32).astype(F16NP))


def _r(x):
    """fp32 -> fp32r RNE rounding (11 explicit mantissa bits)."""
    u = np.ascontiguousarray(x, np.float32).view(np.uint32)
    low = u & np.uint32(0xFFF)
    hi = u >> np.uint32(12)
    carry = (low > 0x800) | ((low == 0x800) & ((hi & 1) == 1))
    return ((hi + carry.astype(np.uint32)) << np.uint32(12)).view(np.float32)


def _slab(W):
    """[128m, D2] weight block -> stationary slab [128 contraction, k*128+m]."""
    n = W.shape[1]
    return np.ascontiguousarray(
        W.T.reshape(n // 128, 128, 128).transpose(1, 0, 2).reshape(128, n))


def _build_program():
    import concourse.bass as bass
    import concourse.bacc as bacc
    import concourse.mybir as mybir
    import concourse.tile as tile

    F32 = mybir.dt.float32
    F32R = mybir.dt.float32r
    BF = mybir.dt.bfloat16
    F16 = mybir.dt.float16
    AF = mybir.ActivationFunctionType
    ALU = mybir.AluOpType

    nc = bacc.Bacc("TRN2", target_bir_lowering=False)

    hinit_in = nc.dram_tensor("hinit", [128, KT * 2 * TG], F32, kind="ExternalInput")
    imgx_in = nc.dram_tensor("imgx", [128, VKT * 2 * TG], F16, kind="ExternalInput")
    projw_in = nc.dram_tensor("projw", [KT, 128, VH], F16, kind="ExternalInput")
    cos_in = nc.dram_tensor("cos_t", [128, S], F32, kind="ExternalInput")
    sin_in = nc.dram_tensor("sin_t", [128, S], F32, kind="ExternalInput")   # sign-folded
    mask_in = nc.dram_tensor("mask6", [6, 128, CH], BF, kind="ExternalInput")
    onesb_in = nc.dram_tensor("onesb", [128, 1], BF, kind="ExternalInput")
    onesh_in = nc.dram_tensor("onesh", [128, 1], F16, kind="ExternalInput")
    identb_in = nc.dram_tensor("identb", [128, 128], BF, kind="ExternalInput")
    lnf_in = nc.dram_tensor("lnf", [128, KT], F32, kind="ExternalInput")
    wqkv_in = [nc.dram_tensor(f"wqkv{l}", [6, 128, D], F16, kind="ExternalInput") for l in range(NL)]
    wo_in = [nc.dram_tensor(f"wo{l}", [KT, 128, QH * 128], F16, kind="ExternalInput") for l in range(NL)]
    wgu_in = [nc.dram_tensor(f"wgu{l}", [2 * FT, 128, D], F16, kind="ExternalInput") for l in range(NL)]
    wd_in = [nc.dram_tensor(f"wd{l}", [KT, 128, FT * 128], F16, kind="ExternalInput") for l in range(NL)]
    out_ext = nc.dram_tensor("out", [128, KT * 2 * TG], F32, kind="ExternalOutput")

    RG = [list(range(NCORES))]

    with tile.TileContext(nc) as tc:
        with tc.tile_pool(name="sb", bufs=1) as sb, \
             tc.tile_pool(name="ps", bufs=1, space="PSUM") as ps, \
             tc.tile_pool(name="dram", bufs=1, space="DRAM") as dram:

            # ---- resident constants ----
            cos_sb = sb.tile([128, S], F32, tag="res_cos", bufs=1)
            sin_sb = sb.tile([128, S], F32, tag="res_sin", bufs=1)
            onesb_sb = sb.tile([128, 1], BF, tag="res_onesb", bufs=1)
            onesh_sb = sb.tile([128, 1], F16, tag="res_onesh", bufs=1)
            identb_sb = sb.tile([128, 128], BF, tag="res_identb", bufs=1)
            lnf_sb = sb.tile([128, KT], F32, tag="res_lnf", bufs=1)
            for t_, i_ in [(cos_sb, cos_in), (sin_sb, sin_in), (onesb_sb, onesb_in), (onesh_sb, onesh_in),
                           (identb_sb, identb_in), (lnf_sb, lnf_in)]:
                nc.sync.dma_start(t_[:], i_[:])
            mask_sb = []
            for j in range(6):
                mt_ = sb.tile([128, CH], BF, tag=f"res_mask{j}", bufs=1, name=f"msk{j}")
                nc.sync.dma_start(mt_[:], mask_in[j])
                mask_sb.append(mt_)
            eps_sb = sb.tile([128, 1], F32, tag="res_eps", bufs=1)
            nb_sb = sb.tile([128, 1], F32, tag="res_nb", bufs=1)
            nc.vector.memset(eps_sb[:], EPS)
            nc.vector.memset(nb_sb[:], EXP_BIAS)

            # ---- persistent activations ----
            hloc = sb.tile([128, KT * 2 * TG], F32, tag="hloc", bufs=1, name="hloc")
            nc.sync.dma_start(hloc[:], hinit_in[:])
            xmega = sb.tile([128, KT * S], F16, tag="xmega", bufs=1, name="xmega")
            mtm = sb.tile([128, FT * S], F16, tag="mtm", bufs=1, name="mtm")

            def hc(b, k):
                return slice(k * 2 * TG + b * TG, k * 2 * TG + (b + 1) * TG)

            # ---- phase 0: token-sharded projector, no collective ----
            # imgx: own tokens' image embeds, [VH rows as 8 k-tiles][2*TG cols],
            # zero cols where own token is not an img token. out += proj_W @ imgx.
            imgx_sb = sb.tile([128, VKT * 2 * TG], F16, tag="imgsb", bufs=1, name="imgx_sb")
            nc.sync.dma_start(imgx_sb[:], imgx_in[:])
            for m in range(KT):
                pw = sb.tile([128, VH], F16, tag="wslab", bufs=3, padded_shape=[128, D],
                             name=f"pw{m}")
                nc.scalar.dma_start(pw[:], projw_in[m])
                pt = ps.tile([128, 2 * TG], F32, tag="ps1", bufs=4, name=f"pj{m}")
                for k in range(VKT):
                    nc.tensor.matmul(pt[:], pw[:, k * 128:(k + 1) * 128],
                                     imgx_sb[:, k * 2 * TG:(k + 1) * 2 * TG],
                                     start=(k == 0), stop=(k == VKT - 1))
                pa = sb.tile([128, 2 * TG], F32, tag="hn", bufs=3, name=f"pa{m}")
                nc.vector.tensor_tensor(pa[:], hloc[:, m * 2 * TG:(m + 1) * 2 * TG],
                                        pt[:], ALU.add)
                nc.scalar.activation(hloc[:, m * 2 * TG:(m + 1) * 2 * TG], pa[:], AF.Copy)

            # ---- helpers ----
            def bcast_row(row_ap, width, nm):
                rd = dram.tile([1, width], F32, tag="rowd", bufs=4, name=f"rd{nm}")
                nc.sync.dma_start(rd[:], row_ap)
                bc = sb.tile([128, width], F32, tag="bc" + str(width), bufs=2, name=f"bct{nm}")
                nc.sync.dma_start(bc[:], rd[:].to_broadcast((128, width)))
                return bc

            def norm_ag(l, site, b, delta, last=False):
                """h update (+delta) for own tokens of batch b, ssq, AllGather x
                (bf16) + 1/rms row. Returns (agout, scout) dram tiles, or the
                output write if last=True."""
                nm = f"{l}{site}{b}"
                agin = [dram.tile([D // 2, TG], F16, tag="agin", bufs=4,
                                  name=f"agi{nm}{h}") for h in range(2)]
                agout = [dram.tile([NCORES, D // 2, TG], F16, tag="agout", bufs=4,
                                   name=f"ago{nm}{h}", addr_space="Shared")
                         for h in range(2)]
                ssq = ps.tile([1, TG], F32, tag="psS", bufs=2, name=f"ssq{nm}")
                if not last:
                    xball = sb.tile([128, KT * TG], F16, tag="xball", bufs=2,
                                    name=f"xba{nm}")
                for k in range(KT):
                    cols = hc(b, k)
                    if delta is not None:
                        dh, dk = (delta[0], k) if k < HK else (delta[1], k - HK)
                        dr = sb.tile([128, TG], F16, tag="drt", bufs=3, name=f"dr{nm}{k}")
                        nc.sync.dma_start(dr[:], dh[dk * 128:(dk + 1) * 128, :])
                        hn = sb.tile([128, TG], F32, tag="hn", bufs=3, name=f"hn{nm}{k}")
                        nc.vector.tensor_tensor(hn[:], hloc[:, cols], dr[:], ALU.add)
                        nc.scalar.activation(hloc[:, cols], hn[:], AF.Copy)
                        hsrc = hn[:]
                    else:
                        hsrc = hloc[:, cols]
                    sqf = sb.tile([128, TG], F32, tag="sqf", bufs=4, name=f"sf{nm}{k}")
                    nc.vector.tensor_tensor(sqf[:], hsrc, hsrc, ALU.mult)
                    if k == 0:
                        acc = sqf
                    else:
                        acc2 = sb.tile([128, TG], F32, tag="sqf", bufs=4,
                                       name=f"ac{nm}{k}")
                        nc.vector.tensor_tensor(acc2[:], acc[:], sqf[:], ALU.add)
                        acc = acc2
                    if not last:
                        nc.vector.tensor_scalar(xball[:, k * TG:(k + 1) * TG], hsrc,
                                                1.0, None, ALU.mult)
                        if k == HK - 1 or k == KT - 1:
                            h = 0 if k < HK else 1
                            nc.sync.dma_start(
                                agin[h][:].rearrange("(k p) t -> p k t", p=128),
                                xball[:, h * HK * TG:(h + 1) * HK * TG])
                            nc.gpsimd.collective_compute(
                                "AllGather", ALU.bypass, replica_groups=RG,
                                ins=[agin[h].opt()], outs=[agout[h].opt()])
                acch = sb.tile([128, TG], F16, tag="sq", bufs=3, name=f"ah{nm}")
                nc.vector.tensor_scalar(acch[:], acc[:], 1.0, None, ALU.mult)
                nc.tensor.matmul(ssq[:], onesh_sb[:], acch[:], start=True, stop=True)
                s_sb = sb.tile([1, TG], F32, tag="scal", bufs=4, name=f"ss{nm}")
                r_sb = sb.tile([1, TG], F32, tag="scal", bufs=4, name=f"sr{nm}")
                nc.scalar.activation(s_sb[:], ssq[:], AF.Sqrt, scale=1.0 / D,
                                     bias=eps_sb[0:1, :])
                nc.vector.reciprocal(r_sb[:], s_sb[:])
                if last:
                    # final norm: scale updated hloc, write own tokens out
                    bcl = bcast_row(r_sb[:], TG, f"f{nm}")
                    for k in range(KT):
                        cols = hc(b, k)
                        ot = sb.tile([128, TG], F32, tag="hn", bufs=3, name=f"ot{nm}{k}")
                        nc.vector.scalar_tensor_tensor(ot[:], hloc[:, cols],
                                                       lnf_sb[:, k:k + 1],
                                                       bcl[:], ALU.mult, ALU.mult)
                        nc.sync.dma_start(out_ext[:, cols], ot[:])
                    return None, None
                scin = dram.tile([1, TG], F32, tag="scin", bufs=2, name=f"sci{nm}")
                scout = dram.tile([1, S], F32, tag="scout", bufs=2, name=f"sco{nm}")
                nc.sync.dma_start(scin[:], r_sb[:])
                nc.gpsimd.collective_compute("AllGather", ALU.bypass, replica_groups=RG,
                                             ins=[scin.opt()], outs=[scout.opt()])
                return agout, scout

            def fill_x(agout, nm):
                """agout [8][D][TG] bf16 -> xmega [128, k*S + 96g+i]."""
                for k in range(KT):
                    ah, ak = (agout[0], k) if k < HK else (agout[1], k - HK)
                    nc.scalar.dma_start(
                        xmega[:, k * S:(k + 1) * S],
                        ah[:, ak * 128:(ak + 1) * 128, :].rearrange("g p t -> p g t"))

            def rs_site(l, site, b, emit_tiles):
                """emit_tiles: iterator of (m, c, psum_tile, scale_or_None).
                Evacuates partials (bf16) into [8][D][TG] and ReduceScatters."""
                nm = f"{l}{site}{b}"
                rsin = [dram.tile([NCORES, D // 2, TG], F16, tag="rsin", bufs=4,
                                  name=f"ri{nm}{h}") for h in range(2)]
                rsout = [dram.tile([D // 2, TG], F16, tag="rsout", bufs=4,
                                   name=f"ro{nm}{h}") for h in range(2)]
                ng = CH // TG
                for (m, c, pt, bcs) in emit_tiles:
                    ev = sb.tile([128, CH], F16, tag="evb", bufs=3, name=f"ev{nm}{m}{c}")
                    if bcs is None:
                        nc.vector.tensor_scalar(ev[:], pt[:], 1.0, None, ALU.mult)
                    else:
                        nc.vector.tensor_tensor(ev[:], pt[:], bcs, ALU.mult)
                    h, mh = (0, m) if m < HK else (1, m - HK)
                    dst = rsin[h][c * ng:(c + 1) * ng, mh * 128:(mh + 1) * 128, :]
                    nc.sync.dma_start(dst.rearrange("g p t -> p g t"), ev[:])
                    if m == HK - 1 and c == NCH - 1:
                        nc.gpsimd.collective_compute(
                            "ReduceScatter", ALU.add, replica_groups=RG,
                            ins=[rsin[0].opt()], outs=[rsout[0].opt()])
                nc.gpsimd.collective_compute("ReduceScatter", ALU.add, replica_groups=RG,
                                             ins=[rsin[1].opt()], outs=[rsout[1].opt()])
                return rsout

            def attn_block(l, b, agout, scout):
                """QKV + rope + attention + Wo partial -> ReduceScatter tile."""
                nm = f"a{l}{b}"
                fill_x(agout, nm)
                bc = bcast_row(scout[:], S, nm)
                qkv = []
                for m in range(6):
                    sl = sb.tile([128, D], F16, tag="wslab", bufs=3, name=f"qs{nm}{m}")
                    nc.scalar.dma_start(sl[:], wqkv_in[l][m])
                    if m < 5:
                        qraw = sb.tile([128, S], F32, tag="t32", bufs=4, name=f"qr{nm}{m}")
                    out_bf = sb.tile([128, S], F16 if m < 5 else BF, tag="qkv", bufs=6, name=f"qo{nm}{m}")
                    for c in range(NCH):
                        pt = ps.tile([128, CH], F32, tag="ps1", bufs=4, name=f"qp{nm}{m}{c}")
                        for k in range(KT):
                            nc.tensor.matmul(
                                pt[:], sl[:, k * 128:(k + 1) * 128],
                                xmega[:, k * S + c * CH: k * S + (c + 1) * CH],
                                start=(k == 0), stop=(k == KT - 1))
                        if m < 5:
                            nc.scalar.activation(qraw[:, c * CH:(c + 1) * CH], pt[:], AF.Copy)
                        else:
                            nc.vector.tensor_tensor(out_bf[:, c * CH:(c + 1) * CH], pt[:],
                                                    bc[:, c * CH:(c + 1) * CH], ALU.mult)
                    if m < 5:
                        qs = sb.tile([128, S], F32, tag="t32", bufs=4, name=f"qh{nm}{m}")
                        nc.scalar.dma_start(qs[0:64, :], qraw[64:128, :])
                        nc.scalar.dma_start(qs[64:128, :], qraw[0:64, :])
                        t2 = sb.tile([128, S], F32, tag="t32", bufs=4, name=f"t2{nm}{m}")
                        nc.vector.tensor_tensor(t2[:], qraw[:], cos_sb[:], ALU.mult)
                        u2 = sb.tile([128, S], F32, tag="t32", bufs=4, name=f"u2{nm}{m}")
                        nc.vector.tensor_tensor(u2[:], qs[:], sin_sb[:], ALU.mult)
                        q3 = sb.tile([128, S], F32, tag="t32", bufs=4, name=f"q3{nm}{m}")
                        nc.vector.tensor_tensor(q3[:], t2[:], u2[:], ALU.add)
                        nc.vector.tensor_tensor(out_bf[:], q3[:], bc[:], ALU.mult)
                    qkv.append(out_bf)

                vtok = []
                for t in range(JT):
                    trp = ps.tile([128, 128], BF, tag="ps1", bufs=4, name=f"vt{nm}{t}")
                    nc.tensor.transpose(trp[:], qkv[5][:, t * 128:(t + 1) * 128],
                                        identb_sb[:])
                    vt = sb.tile([128, 128], BF, tag="vtok", bufs=6, name=f"vk{nm}{t}")
                    nc.scalar.activation(vt[:], trp[:], AF.Copy)
                    vtok.append(vt)

                amega = sb.tile([128, QH * S], F16, tag="amega", bufs=1, name=f"am{nm}")
                ksb = qkv[4]
                for hh in range(QH):
                    qh_t = qkv[hh]
                    for c in range(NCH):
                        njt = 3 * (c + 1)
                        ap_ps = ps.tile([128, CH], F32, tag="psA", bufs=2, name=f"ap{nm}{hh}{c}")
                        ss_ps = ps.tile([1, CH], F32, tag="psS", bufs=2, name=f"sm{nm}{hh}{c}")
                        for jt in range(njt):
                            sc = ps.tile([128, CH], F32, tag="ps1", bufs=4,
                                         name=f"sc{nm}{hh}{c}{jt}")
                            nc.tensor.matmul(sc[:], ksb[:, jt * 128:(jt + 1) * 128],
                                             qh_t[:, c * CH:(c + 1) * CH],
                                             start=True, stop=True)
                            et = sb.tile([128, CH], BF, tag="expT", bufs=4,
                                         name=f"et{nm}{hh}{c}{jt}")
                            if jt >= 3 * c:
                                madd = sb.tile([128, CH], F32, tag="madd", bufs=3,
                                               name=f"md{nm}{hh}{c}{jt}")
                                nc.vector.tensor_tensor(madd[:], sc[:], mask_sb[jt][:],
                                                        ALU.add)
                                nc.scalar.activation(et[:], madd[:], AF.Exp, scale=ISQ,
                                                     bias=nb_sb[:])
                            else:
                                nc.scalar.activation(et[:], sc[:], AF.Exp, scale=ISQ,
                                                     bias=nb_sb[:])
                            nc.tensor.matmul(ss_ps[:], onesb_sb[:], et[:],
                                             start=(jt == 0), stop=(jt == njt - 1))
                            nc.tensor.matmul(ap_ps[:], vtok[jt][:], et[:],
                                             start=(jt == 0), stop=(jt == njt - 1))
                        rec = sb.tile([1, CH], F32, tag="scal", bufs=4, name=f"rc{nm}{hh}{c}")
                        nc.vector.reciprocal(rec[:], ss_ps[:])
                        rbc = bcast_row(rec[:], CH, f"r{nm}{hh}{c}")
                        nc.vector.tensor_tensor(
                            amega[:, hh * S + c * CH: hh * S + (c + 1) * CH],
                            ap_ps[:], rbc[:], ALU.mult)

                def wo_tiles():
                    for m in range(KT):
                        sl = sb.tile([128, QH * 128], F16, tag="wslab", bufs=3,
                                     padded_shape=[128, D], name=f"wos{nm}{m}")
                        nc.scalar.dma_start(sl[:], wo_in[l][m])
                        for c in range(NCH):
                            pt = ps.tile([128, CH], F32, tag="ps1", bufs=4,
                                         name=f"op{nm}{m}{c}")
                            for k in range(QH):
                                nc.tensor.matmul(
                                    pt[:], sl[:, k * 128:(k + 1) * 128],
                                    amega[:, k * S + c * CH: k * S + (c + 1) * CH],
                                    start=(k == 0), stop=(k == QH - 1))
                            yield (m, c, pt, None)
                return rs_site(l, 'a', b, wo_tiles())

            def mlp_block(l, b, agout, scout):
                nm = f"m{l}{b}"
                fill_x(agout, nm)
                bc = bcast_row(scout[:], S, nm)
                for j in range(FT):
                    gsb = sb.tile([128, S], F32, tag="t32", bufs=4, name=f"gs{nm}{j}")
                    usb = sb.tile([128, S], F32, tag="t32", bufs=4, name=f"us{nm}{j}")
                    for gu in range(2):
                        sl = sb.tile([128, D], F16, tag="wslab", bufs=3, name=f"gsl{nm}{j}{gu}")
                        nc.scalar.dma_start(sl[:], wgu_in[l][2 * j + gu])
                        dst = gsb if gu == 0 else usb
                        for c in range(NCH):
                            pt = ps.tile([128, CH], F32, tag="ps1", bufs=4,
                                         name=f"g{nm}{j}{gu}{c}")
                            for k in range(KT):
                                nc.tensor.matmul(
                                    pt[:], sl[:, k * 128:(k + 1) * 128],
                                    xmega[:, k * S + c * CH: k * S + (c + 1) * CH],
                                    start=(k == 0), stop=(k == KT - 1))
                            nc.vector.tensor_tensor(dst[:, c * CH:(c + 1) * CH], pt[:],
                                                    bc[:, c * CH:(c + 1) * CH], ALU.mult)
                    sil = sb.tile([128, S], F32, tag="t32", bufs=4, name=f"si{nm}{j}")
                    nc.scalar.activation(sil[:], gsb[:], AF.Silu)
                    nc.vector.tensor_tensor(mtm[:, j * S:(j + 1) * S], sil[:], usb[:],
                                            ALU.mult)

                def wd_tiles():
                    for m in range(KT):
                        sl = sb.tile([128, FT * 128], F16, tag="wslab", bufs=3,
                                     padded_shape=[128, D], name=f"wds{nm}{m}")
                        nc.scalar.dma_start(sl[:], wd_in[l][m])
                        for c in range(NCH):
                            pt = ps.tile([128, CH], F32, tag="ps1", bufs=4,
                                         name=f"dp{nm}{m}{c}")
                            for k in range(FT):
                                nc.tensor.matmul(
                                    pt[:], sl[:, k * 128:(k + 1) * 128],
                                    mtm[:, k * S + c * CH: k * S + (c + 1) * CH],
                                    start=(k == 0), stop=(k == FT - 1))
                            yield (m, c, pt, None)
                return rs_site(l, 'm', b, wd_tiles())

            # ---- main schedule ----
            rs_prev = [None, None]          # pending MLP RS from previous layer
            for l in range(NL):
                ags = [None, None]
                for b in range(B):
                    ags[b] = norm_ag(l, 'a', b, rs_prev[b])
                rs_a = [None, None]
                for b in range(B):
                    rs_a[b] = attn_block(l, b, ags[b][0], ags[b][1])
                for b in range(B):
                    ags[b] = norm_ag(l, 'm', b, rs_a[b])
                for b in range(B):
                    rs_prev[b] = mlp_block(l, b, ags[b][0], ags[b][1])
                    if l == NL - 1:
                        norm_ag(NL, 'f', b, rs_prev[b], last=True)

    nc.compile()
    return nc


def _host_prep(inputs):
    I = {k: np.asarray(v) for k, v in inputs.items()}

    def fold(W, A, Bm, lnw=None):
        Wf = W.astype(np.float32) + np.float32(SCALE) * (
            Bm.astype(np.float32) @ A.astype(np.float32))
        if lnw is not None:
            Wf = Wf * lnw.astype(np.float32)[None, :]
        return Wf

    ids = np.asarray(I['input_ids'], np.int64)
    embed = I['embed'].astype(np.float32)
    proj_b = I['proj_b'].astype(np.float32)

    # initial h per batch: [B, S, D]; img positions get proj bias (GEMM adds the rest)
    h0 = np.empty((B, S, D), np.float32)
    h0[:, :NIMG, :] = proj_b[None, None, :]
    h0[:, NIMG:, :] = embed[ids]

    inv = 1.0 / (10000.0 ** (np.arange(0, HD, 2, dtype=np.float64) / HD))
    ang = np.arange(S, dtype=np.float64)[:, None] * inv[None, :]
    cosT = np.ascontiguousarray(np.concatenate([np.cos(ang), np.cos(ang)], 1).T).astype(np.float32)
    sinT = np.ascontiguousarray(np.concatenate([-np.sin(ang), np.sin(ang)], 1).T).astype(np.float32)

    mask6 = np.zeros((6, 128, CH), np.float32)
    for jt in range(6):
        c = 0 if jt < 3 else 1
        jj = np.arange(jt * 128, (jt + 1) * 128)[:, None]
        ii = np.arange(c * CH, (c + 1) * CH)[None, :]
        mask6[jt] = np.where(jj <= ii, 0.0, MASK_NEG)

    imgT = I['image_embeds'].astype(np.float32)     # [B, NIMG, VH]
    projW = I['proj_W'].astype(np.float32)          # [D, VH]
    # proj slabs: [m][kk, vh_k*128+mm] = projW[m*128+mm, vh_k*128+kk]
    projw_slab = np.stack([_slab(projW[m * 128:(m + 1) * 128, :]) for m in range(KT)])

    shared = dict(
        cos_t=cosT, sin_t=sinT, mask6=_bf(mask6),
        onesb=_bf(np.ones((128, 1), np.float32)),
        onesh=_h(np.ones((128, 1), np.float32)),
        identb=_bf(np.eye(128, dtype=np.float32)),
        lnf=np.ascontiguousarray(I['ln_f'].astype(np.float32).reshape(KT, 128).T),
        projw=_h(projw_slab),
    )

    per_core = [dict(shared) for _ in range(NCORES)]
    for r in range(NCORES):
        t0 = r * TG
        # hinit [128, k*2TG + b*TG + i] = h0[b, t0+i, k*128+p]
        own = h0[:, t0:t0 + TG, :]                       # [B, TG, D]
        per_core[r]["hinit"] = np.ascontiguousarray(
            own.reshape(B, TG, KT, 128).transpose(3, 2, 0, 1).reshape(128, KT * 2 * TG))
        # imgx: [128, vk*2TG + b*TG + i] = img_embed[b, t0+i, vk*128+p] or 0
        ix = np.zeros((B, TG, VH), np.float32)
        w = max(0, min(TG, NIMG - t0))
        if w > 0:
            ix[:, :w, :] = imgT[:, t0:t0 + w, :]
        per_core[r]["imgx"] = _h(np.ascontiguousarray(
            ix.reshape(B, TG, VKT, 128).transpose(3, 2, 0, 1).reshape(128, VKT * 2 * TG)))

    for l in range(NL):
        Wq = fold(I['Wq'][l], I['Aq'][l], I['Bq'][l], I['ln1'][l])
        Wk = fold(I['Wk'][l], I['Ak'][l], I['Bk'][l], I['ln1'][l])
        Wv = fold(I['Wv'][l], I['Av'][l], I['Bv'][l], I['ln1'][l])
        Wo = fold(I['Wo'][l], I['Ao'][l], I['Bo'][l])
        Wg = fold(I['Wg'][l], I['Ag'][l], I['Bg'][l], I['ln2'][l])
        Wu = fold(I['Wu'][l], I['Au'][l], I['Bu'][l], I['ln2'][l])
        Wd = fold(I['Wd'][l], I['Ad'][l], I['Bd'][l])
        for r in range(NCORES):
            wl = np.vstack([Wq[r * 512:(r + 1) * 512],
                            Wk[r * HD:(r + 1) * HD],
                            Wv[r * HD:(r + 1) * HD]])          # [768, D]
            per_core[r][f"wqkv{l}"] = _h(np.stack(
                [_slab(wl[m * 128:(m + 1) * 128, :]) for m in range(6)]))
            wo_l = Wo[:, r * 512:(r + 1) * 512]                # [D, 512]
            per_core[r][f"wo{l}"] = _h(np.stack(
                [_slab(wo_l[m * 128:(m + 1) * 128, :]) for m in range(KT)]))
            gu = np.empty((2 * FT * 128, D), np.float32)
            gsh = Wg[r * FT * 128:(r + 1) * FT * 128]
            ush = Wu[r * FT * 128:(r + 1) * FT * 128]
            for j in range(FT):
                gu[(2 * j) * 128:(2 * j + 1) * 128] = gsh[j * 128:(j + 1) * 128]
                gu[(2 * j + 1) * 128:(2 * j + 2) * 128] = ush[j * 128:(j + 1) * 128]
            per_core[r][f"wgu{l}"] = _h(np.stack(
                [_slab(gu[m * 128:(m + 1) * 128, :]) for m in range(2 * FT)]))
            wd_l = Wd[:, r * FT * 128:(r + 1) * FT * 128]      # [D, 1792]
            per_core[r][f"wd{l}"] = _h(np.stack(
                [_slab(wd_l[m * 128:(m + 1) * 128, :]) for m in range(KT)]))
    return per_core


def kernel(**inputs):
    global _PROGRAM
    from concourse.bass_utils import run_bass_kernel_spmd

    in_maps = _host_prep(inputs)
    if _PROGRAM is None:
        _PROGRAM = _build_program()
    res = None
    for attempt in range(3):
        try:
            res = run_bass_kernel_spmd(_PROGRAM, in_maps, list(range(NCORES)))
            break
        except Exception as e:
            if attempt == 2 or 'UNAVAILABLE' not in str(type(e).__name__) + str(e):
                raise
    out = np.empty((B, S, D), np.float32)
    for r in range(NCORES):
        o = np.asarray(res.results[r]["out"], np.float32)      # [128, KT*2*TG]
        o = o.reshape(128, KT, B, TG).transpose(2, 3, 1, 0).reshape(B, TG, D)
        out[:, r * TG:(r + 1) * TG, :] = o
    return out
